# revision 3
# baseline (speedup 1.0000x reference)
"""GroupLinear (MoE routing) Trainium2 kernel.

Problem: x [8192, 1024] f32, indices [8192] int64 in [0,8),
W [8*2048, 1024] f32, b [8*2048] f32.
out[n] = x[n] @ W[g*2048:(g+1)*2048].T + b[g*2048:(g+1)*2048],  g = indices[n].

Strategy: expert-parallel across the 8 NeuronCores. Core g owns group g's
weight slice only (8MB instead of the full 64MB), and processes exactly the
rows routed to group g. Row routing (argsort of indices) happens on host;
the device kernel is a dense [C_pad, 1024] @ [1024, 2048] + bias matmul in
float32r (full PE rate, near-fp32 precision).

Device-side layout (per core):
  xT   [1024, C_pad]  : gathered x rows, transposed (K-major for matmul lhsT)
  wT   [1024, 2048]   : W_g transposed (K-major for matmul rhs)
  bias [1, 2048]
  out  [C_pad, 2048]  : rows in routed order; host scatters back.

Matmul mapping: out[c_blk(128), o_blk(512)] = sum_k xT[k,c].T @ wT[k,o]
  lhsT = xT[kc*128:(kc+1)*128, c_blk]   (stationary, [128,128])
  rhs  = wT[kc*128:(kc+1)*128, o_blk]   (moving, [128,512])
Bias is added during PSUM->SBUF eviction on the Vector engine, from a
partition-broadcast bias tile.
"""

import os
import sys

sys.path.insert(0, "/opt/trn_rl_repo")

import numpy as np

import concourse.bass as bass
import concourse.bacc as bacc
import concourse.mybir as mybir
import concourse.tile as tile
from concourse.bass_utils import run_bass_kernel_spmd

N = 8192
IN_F = 1024
OUT_F = 2048
G = 8
NCORES = 8
P = 128
NB_SZ = 512  # matmul moving-dim / PSUM bank free size (fp32)

LAST_EXEC_NS = None
LAST_RESULTS = None

_nc_cache = {}


def _build_nc(c_pad: int):
    """Build the per-core Bass program for C_pad routed rows."""
    assert c_pad % P == 0
    kc_n = IN_F // P   # 8 k-chunks
    nb_n = OUT_F // NB_SZ  # 4 output-feature blocks
    mb_n = c_pad // P  # row blocks

    nc = bacc.Bacc("TRN2", target_bir_lowering=False, debug=False)
    f32r = mybir.dt.float32r

    xT = nc.dram_tensor("xT", [IN_F, c_pad], f32r, kind="ExternalInput")
    wT = nc.dram_tensor("wT", [IN_F, OUT_F], f32r, kind="ExternalInput")
    bias = nc.dram_tensor("bias", [1, OUT_F], mybir.dt.float32, kind="ExternalInput")
    out = nc.dram_tensor("out", [c_pad, OUT_F], mybir.dt.float32, kind="ExternalOutput")

    with tile.TileContext(nc) as tc:
        with (
            tc.tile_pool(name="wp", bufs=1) as wp,
            tc.tile_pool(name="xp", bufs=1) as xp,
            tc.tile_pool(name="bp", bufs=1) as bp,
            tc.tile_pool(name="op", bufs=4) as op,
            tc.tile_pool(name="pp", bufs=8, space="PSUM") as pp,
        ):
            bias_sb = bp.tile([P, OUT_F], mybir.dt.float32, tag="bias")
            nc.sync.dma_start(bias_sb[:], bias[0:1, :].to_broadcast((P, OUT_F)))

            # Persistent SBUF tiles; emitted in the order compute consumes
            # them so the Tile scheduler's DMA stream stays ahead of PE.
            x_sb = [None] * kc_n
            w_sb = [[None] * nb_n for _ in range(kc_n)]
            for kc in range(kc_n):
                x_sb[kc] = xp.tile([P, c_pad], f32r, name=f"x{kc}", tag=f"x{kc}")
                nc.sync.dma_start(x_sb[kc][:], xT[kc * P:(kc + 1) * P, :])
                w_sb[kc][0] = wp.tile([P, NB_SZ], f32r, name=f"w{kc}_0", tag=f"w{kc}_0")
                nc.sync.dma_start(w_sb[kc][0][:], wT[kc * P:(kc + 1) * P, 0:NB_SZ])
            for nb in range(1, nb_n):
                for kc in range(kc_n):
                    w_sb[kc][nb] = wp.tile(
                        [P, NB_SZ], f32r,
                        name=f"w{kc}_{nb}", tag=f"w{kc}_{nb}",
                    )
                    nc.sync.dma_start(
                        w_sb[kc][nb][:],
                        wT[kc * P:(kc + 1) * P, nb * NB_SZ:(nb + 1) * NB_SZ],
                    )

            for nb in range(nb_n):
                for mb in range(mb_n):
                    psum = pp.tile([P, NB_SZ], mybir.dt.float32, tag="psum")
                    for kc in range(kc_n):
                        nc.tensor.matmul(
                            psum[:],
                            x_sb[kc][:, mb * P:(mb + 1) * P],
                            w_sb[kc][nb][:],
                            start=(kc == 0),
                            stop=(kc == kc_n - 1),
                        )
                    ot = op.tile([P, NB_SZ], mybir.dt.float32, tag="ot")
                    nc.vector.tensor_add(
                        ot[:], psum[:], bias_sb[:, nb * NB_SZ:(nb + 1) * NB_SZ]
                    )
                    nc.sync.dma_start(
                        out[mb * P:(mb + 1) * P, nb * NB_SZ:(nb + 1) * NB_SZ], ot[:]
                    )

    nc.compile()
    return nc


def _get_nc(c_pad: int):
    nc = _nc_cache.get(c_pad)
    if nc is None:
        nc = _build_nc(c_pad)
        _nc_cache[c_pad] = nc
    return nc


def kernel(x, indices, W, b):
    global LAST_EXEC_NS, LAST_RESULTS

    x = np.ascontiguousarray(np.asarray(x, dtype=np.float32))
    W = np.ascontiguousarray(np.asarray(W, dtype=np.float32))
    b = np.asarray(b, dtype=np.float32)
    idx = np.asarray(indices).astype(np.int64)

    order = np.argsort(idx, kind="stable")
    counts = np.bincount(idx, minlength=G)
    offs = np.zeros(G + 1, dtype=np.int64)
    np.cumsum(counts, out=offs[1:])

    c_pad = max(P, int(-(-counts.max() // P)) * P)
    nc = _get_nc(c_pad)

    rows = [order[offs[g]:offs[g + 1]] for g in range(G)]
    in_maps = []
    for g in range(G):
        xg = np.zeros((IN_F, c_pad), dtype=np.float32)
        cg = int(counts[g])
        if cg:
            xg[:, :cg] = x[rows[g]].T
        wtg = np.ascontiguousarray(W[g * OUT_F:(g + 1) * OUT_F, :].T)
        bg = np.ascontiguousarray(b[g * OUT_F:(g + 1) * OUT_F]).reshape(1, OUT_F)
        in_maps.append({"xT": xg, "wT": wtg, "bias": bg})

    trace = bool(int(os.environ.get("KERNEL_TRACE", "0")))
    res = run_bass_kernel_spmd(nc, in_maps, list(range(NCORES)), trace=trace)
    LAST_EXEC_NS = res.exec_time_ns
    LAST_RESULTS = res

    out = np.empty((N, OUT_F), dtype=np.float32)
    for g in range(G):
        cg = int(counts[g])
        if cg:
            out[rows[g]] = res.results[g]["out"][:cg]
    return out
